# revision 10
# baseline (speedup 1.0000x reference)
"""Trainium2 Bass kernel for GQA attention block (nn_Attention_52115133170131).

Strategy (8 NeuronCores, tensor-parallel over heads):
  - Core c owns q-heads 4c..4c+3 and kv-head c (wq/wk/wv column-sharded,
    head-dim pairs permuted evens-first so RoPE works on contiguous
    partition halves).
  - Per core: fused QKV projection (K=4096 float32r matmuls), RoPE on q/k,
    causal attention in transposed (feature-major) space: exp on ScalarE,
    softmax denominator via a ones-column matmul accumulated in PSUM,
    PV accumulated in PSUM, normalization via reciprocal + partition
    broadcast.
  - Per-batch AllToAll redistributes attention output from head-sharded to
    token-sharded (bf16 payload); each core then runs the full wo matmul
    for its 256 tokens of that batch (full contraction locally, no partial
    sums).  Batch-0's AllToAll and wo sweep overlap batch-1's attention.
  - Host gathers: concat token slices (out), stack kv-head shards
    (new_k/new_v; device writes them feature-major, host transposes).

QKV/attention matmuls run in float32r (full PE rate at N>=256, ~1.5e-4
rel err).  The wo projection runs in bf16 (WO_BF16) which only affects
`out` (~1e-3 rel err) and halves the collective payload.
"""
from contextlib import ExitStack

import numpy as np

import concourse.bacc as bacc
import concourse.mybir as mybir
import concourse.tile as tile
from concourse.bass_utils import run_bass_kernel_spmd

# problem constants
N_CORES = 8
B = 2
S = 2048
DM = 4096            # model dim
NH = 32              # query heads
NKV = 8              # kv heads
HD = 128             # head dim
HQ = NH // N_CORES   # q heads per core = 4
T = B * S            # 4096 tokens
TB = S // N_CORES    # tokens per core per batch = 256
ROPE_THETA = 10000.0
SCALE = float(1.0 / np.sqrt(np.float32(HD)))

P = 128
KO = DM // P         # 32 contraction k-tiles
NTC = T // 512       # 8 projection t-chunks
MW = HQ * HD + 2 * HD  # 768 fused qkv out dim per core

F32R = mybir.dt.float32r
F32 = mybir.dt.float32
BF16 = mybir.dt.bfloat16

WO_BF16 = True       # wo matmul + AllToAll payload in bf16
SPLIT = True         # per-batch A2A overlapped with attention/wo
XT_BUFS = 12
QQ_BUFS = 3
EE_BUFS = 5
WT_BUFS = 6

_PERM = np.concatenate([np.arange(0, HD, 2), np.arange(1, HD, 2)])  # evens-first

_CACHE = {}


def _build_nc(reps=1):
    a2a_dt = BF16 if WO_BF16 else F32R

    nc = bacc.Bacc("TRN2", target_bir_lowering=False)

    xT_d = nc.dram_tensor("xT", [DM, T], F32R, kind="ExternalInput")
    wT_d = nc.dram_tensor("wT", [KO, P, MW], F32R, kind="ExternalInput")
    woT_d = nc.dram_tensor("woT", [DM, DM], a2a_dt, kind="ExternalInput")
    cs_d = nc.dram_tensor("cs", [P, S], F32, kind="ExternalInput")
    sn_d = nc.dram_tensor("sn", [P, S], F32, kind="ExternalInput")
    masks_d = nc.dram_tensor("masks", [P, 2 * 512], F32R, kind="ExternalInput")
    ones_d = nc.dram_tensor("ones", [P, 1], F32R, kind="ExternalInput")
    ident_d = nc.dram_tensor("ident", [P, P], F32R, kind="ExternalInput")

    out_d = nc.dram_tensor("out_c", [2 * TB, DM], F32, kind="ExternalOutput")
    newk_d = nc.dram_tensor("newkT", [B, P, S], F32, kind="ExternalOutput")
    newv_d = nc.dram_tensor("newvT", [B, P, S], F32, kind="ExternalOutput")

    qsp_d = nc.dram_tensor("qsp", [B, HQ, P, S], F32R)
    a2a_in_d = [nc.dram_tensor(f"a2a_in{b}", [N_CORES, 512, TB], a2a_dt)
                for b in range(B)]
    a2a_out_d = [nc.dram_tensor(f"a2a_out{b}", [N_CORES, 512, TB], a2a_dt)
                 for b in range(B)]

    with tile.TileContext(nc) as tc:
        with ExitStack() as consts:
            cpool = consts.enter_context(tc.tile_pool(name="consts", bufs=1))
            cs = cpool.tile([P, S], F32)
            sn = cpool.tile([P, S], F32)
            masks = cpool.tile([P, 2 * 512], F32R)
            ones = cpool.tile([P, 1], F32R)
            ident = cpool.tile([P, P], F32R)
            for t, d in [(cs, cs_d), (sn, sn_d), (masks, masks_d),
                         (ones, ones_d), (ident, ident_d)]:
                nc.sync.dma_start(out=t[:], in_=d[:])

            # resident attention operands
            kvpool = consts.enter_context(tc.tile_pool(name="kv", bufs=1))
            kT_rot = [kvpool.tile([P, S], F32R, name=f"kTrot{b}") for b in range(B)]
            v_tok = [[kvpool.tile([P, P], F32R, name=f"vtok{b}_{j}")
                      for j in range(S // P)] for b in range(B)]

            for _rep in range(reps):
                _emit_body(nc, tc, cs, sn, masks, ones, ident, kT_rot, v_tok,
                           xT_d, wT_d, woT_d, out_d, newk_d, newv_d,
                           qsp_d, a2a_in_d, a2a_out_d, a2a_dt)
    nc.compile()
    return nc


def _emit_body(nc, tc, cs, sn, masks, ones, ident, kT_rot, v_tok,
               xT_d, wT_d, woT_d, out_d, newk_d, newv_d,
               qsp_d, a2a_in_d, a2a_out_d, a2a_dt):
    # ---------------- phase B: fused qkv projection + rope ------------------
    with ExitStack() as ph:
        wpool = ph.enter_context(tc.tile_pool(name="wq", bufs=1))
        W = []
        for ko in range(KO):
            w = wpool.tile([P, MW], F32R, name=f"w{ko}")
            nc.sync.dma_start(out=w[:], in_=wT_d[ko])
            W.append(w)

        xpool = ph.enter_context(tc.tile_pool(name="xt", bufs=XT_BUFS))
        pps = ph.enter_context(tc.tile_pool(name="projps", bufs=1, space="PSUM"))
        tpps = ph.enter_context(tc.tile_pool(name="tpps", bufs=2, space="PSUM"))
        ev = ph.enter_context(tc.tile_pool(name="evict", bufs=2))

        def rope(dst, src, col0, n):
            # dst[0:64]   = src[0:64]*cs - src[64:128]*sn   (on dst lanes)
            # dst[64:128] = src[0:64]*sn + src[64:128]*cs
            A = ev.tile([P, n], F32, name="ropeA")
            Bt = ev.tile([P, n], F32, name="ropeB")
            Bs = ev.tile([P, n], F32, name="ropeBs")
            nc.vector.tensor_mul(A[:], src[:], cs[:, col0:col0 + n])
            nc.vector.tensor_mul(Bt[:], src[:], sn[:, col0:col0 + n])
            nc.sync.dma_start(out=Bs[0:64, :], in_=Bt[64:128, :])
            nc.sync.dma_start(out=Bs[64:128, :], in_=Bt[0:64, :])
            nc.vector.tensor_sub(dst[0:64, :], A[0:64, :], Bs[0:64, :])
            nc.vector.tensor_add(dst[64:128, :], Bs[64:128, :], A[64:128, :])

        for tcn in range(NTC):
            b, tci = tcn // 4, tcn % 4
            col0 = 512 * tci
            ps = [pps.tile([P, 512], F32, name=f"pp{m}") for m in range(6)]
            for ko in range(KO):
                xt = xpool.tile([P, 512], F32R, name="xt")
                nc.sync.dma_start(
                    out=xt[:], in_=xT_d[P * ko:P * (ko + 1),
                                        512 * tcn:512 * (tcn + 1)])
                for m in range(6):
                    nc.tensor.matmul(
                        ps[m][:], W[ko][:, P * m:P * (m + 1)], xt[:],
                        start=(ko == 0), stop=(ko == KO - 1))
            # q heads: rope -> spill
            for m in range(HQ):
                qrt = ev.tile([P, 512], F32R, name="qrt")
                rope(qrt, ps[m], col0, 512)
                nc.sync.dma_start(out=qsp_d[b, m, :, col0:col0 + 512], in_=qrt[:])
            # k: new_k (pre-rope) + rope into resident kT_rot
            nkst = ev.tile([P, 512], F32, name="nkst")
            nc.vector.tensor_copy(nkst[:], ps[4][:])
            nc.sync.dma_start(out=newk_d[b][:, col0:col0 + 512], in_=nkst[:])
            rope(kT_rot[b][:, col0:col0 + 512], ps[4], col0, 512)
            # v: new_v + transpose to token-major tiles
            vst = ev.tile([P, 512], F32R, name="vst")
            nc.vector.tensor_copy(vst[:], ps[5][:])
            nvst = ev.tile([P, 512], F32, name="nvst")
            nc.vector.tensor_copy(nvst[:], ps[5][:])
            nc.sync.dma_start(out=newv_d[b][:, col0:col0 + 512], in_=nvst[:])
            for j in range(4):
                tp = tpps.tile([P, P], F32R, name="tp")
                nc.tensor.transpose(tp[:], vst[:, P * j:P * (j + 1)], ident[:])
                nc.vector.tensor_copy(v_tok[b][4 * tci + j][:], tp[:])

    # ---------------- phase C/D: attention, per-batch A2A, wo ---------------
    def attn_batch(ph, b):
        # software-pipelined inner loop: PV/den for tile j are emitted SKEW
        # scores-tiles later, so the exp (ACT) + mask (DVE) latency of et(j)
        # is covered by PE work instead of stalling the PE queue.
        SKEW = 2
        qpool, spool, opool, dpool, epool, npool = ph
        for h in range(HQ):
            for qc in range(4):
                q0 = 512 * qc
                ktmax = 4 * (qc + 1)
                qt = qpool.tile([P, 512], F32R, name="qt")
                nc.sync.dma_start(out=qt[:], in_=qsp_d[b, h, :, q0:q0 + 512])
                otps = opool.tile([P, 512], F32, name="otps")
                dnps = dpool.tile([1, 512], F32, name="dnps")

                def tile_window(j):
                    # causal: tile j only needs q columns >= 128*jj; clamp
                    # the window to >=256 cols (f32r full-rate floor)
                    jj = j - 4 * qc
                    if jj >= 1:
                        npr = max(512 - 128 * jj, 256)
                        n0 = 512 - npr
                    else:
                        n0, npr = 0, 512
                    return jj, n0, npr

                ets = {}

                def emit_scores(j):
                    jj, n0, npr = tile_window(j)
                    sps = spool.tile([P, 512], F32, name="sps")
                    nc.tensor.matmul(sps[:, n0:n0 + npr],
                                     kT_rot[b][:, P * j:P * (j + 1)],
                                     qt[:, n0:n0 + npr], start=True, stop=True)
                    et = epool.tile([P, 512], F32R, name="et")
                    nc.scalar.activation(et[:, n0:n0 + npr], sps[:, n0:n0 + npr],
                                         mybir.ActivationFunctionType.Exp,
                                         scale=SCALE)
                    if jj >= 0:
                        o = jj - n0 // P
                        nc.vector.tensor_mul(
                            et[:, n0:n0 + npr], et[:, n0:n0 + npr],
                            masks[:, 512 * o:512 * o + npr])
                    ets[j] = (et, n0, npr)

                def emit_pv_den(j):
                    et, n0, npr = ets.pop(j)
                    nc.tensor.matmul(otps[:, n0:n0 + npr], v_tok[b][j][:],
                                     et[:, n0:n0 + npr],
                                     start=(j == 0), stop=(j == ktmax - 1))
                    nc.tensor.matmul(dnps[:, n0:n0 + npr], ones[:],
                                     et[:, n0:n0 + npr],
                                     start=(j == 0), stop=(j == ktmax - 1))

                for j in range(ktmax):
                    emit_scores(j)
                    if j >= SKEW:
                        emit_pv_den(j - SKEW)
                for j in range(max(0, ktmax - SKEW), ktmax):
                    emit_pv_den(j)

                rc = npool.tile([1, 512], F32, name="rc")
                nc.vector.reciprocal(rc[:], dnps[:])
                rb = npool.tile([P, 512], F32, name="rb")
                nc.gpsimd.partition_broadcast(rb[:], rc[:])
                at = npool.tile([P, 512], a2a_dt, name="at")
                nc.vector.tensor_mul(at[:], otps[:], rb[:])
                nc.sync.dma_start(
                    out=a2a_in_d[b][2 * qc, HD * h:HD * (h + 1), :],
                    in_=at[:, 0:TB])
                nc.sync.dma_start(
                    out=a2a_in_d[b][2 * qc + 1, HD * h:HD * (h + 1), :],
                    in_=at[:, TB:512])

    def wo_batch(ph, b):
        lpool, wopool, wps, oev = ph
        lt = []
        for f in range(KO):
            t_ = lpool.tile([P, TB], a2a_dt, name=f"lt{b}_{f}")
            nc.sync.dma_start(
                out=t_[:],
                in_=a2a_out_d[b][f // 4, P * (f % 4):P * (f % 4 + 1), :])
            lt.append(t_)
        for n in range(DM // 512):
            pso = [wps.tile([P, 512], F32, name=f"wps{i}") for i in range(TB // P)]
            for f in range(KO):
                wt = wopool.tile([P, 512], a2a_dt, name="wt")
                nc.sync.dma_start(
                    out=wt[:], in_=woT_d[P * f:P * (f + 1),
                                         512 * n:512 * (n + 1)])
                for t_ in range(TB // P):
                    nc.tensor.matmul(pso[t_][:], lt[f][:, P * t_:P * (t_ + 1)],
                                     wt[:], start=(f == 0), stop=(f == KO - 1))
            for t_ in range(TB // P):
                ot = oev.tile([P, 512], F32, name="ot")
                nc.vector.tensor_copy(ot[:], pso[t_][:])
                r0 = TB * b + P * t_
                nc.sync.dma_start(
                    out=out_d[r0:r0 + P, 512 * n:512 * (n + 1)], in_=ot[:])

    with ExitStack() as ph:
        attn_pools = (
            ph.enter_context(tc.tile_pool(name="qq", bufs=QQ_BUFS)),
            ph.enter_context(tc.tile_pool(name="sps", bufs=3, space="PSUM")),
            ph.enter_context(tc.tile_pool(name="ops", bufs=2, space="PSUM")),
            ph.enter_context(tc.tile_pool(name="dps", bufs=1, space="PSUM")),
            ph.enter_context(tc.tile_pool(name="ee", bufs=EE_BUFS)),
            ph.enter_context(tc.tile_pool(name="nrm", bufs=2)),
        )
        wo_pools = (
            ph.enter_context(tc.tile_pool(name="lt", bufs=1)),
            ph.enter_context(tc.tile_pool(name="wo", bufs=WT_BUFS)),
            ph.enter_context(tc.tile_pool(name="wops", bufs=1, space="PSUM")),
            ph.enter_context(tc.tile_pool(name="oev", bufs=3)),
        )
        rg = [list(range(N_CORES))]

        def a2a(b):
            nc.gpsimd.collective_compute(
                "AllToAll", mybir.AluOpType.bypass,
                ins=[a2a_in_d[b][:]], outs=[a2a_out_d[b][:]], replica_groups=rg)

        if SPLIT:
            attn_batch(attn_pools, 0)
            a2a(0)
            attn_batch(attn_pools, 1)
            wo_batch(wo_pools, 0)
            a2a(1)
            wo_batch(wo_pools, 1)
        else:
            attn_batch(attn_pools, 0)
            attn_batch(attn_pools, 1)
            a2a(0)
            a2a(1)
            wo_batch(wo_pools, 0)
            wo_batch(wo_pools, 1)


def _host_prep(x, wq, wk, wv, wo):
    import ml_dtypes

    # rope tables exactly as the reference computes them (fp32 ops)
    inv_freq = (1.0 / (ROPE_THETA ** (np.arange(0, HD, 2, dtype=np.float32) / HD))
                ).astype(np.float32)
    ang = np.arange(S, dtype=np.float32)[:, None] * inv_freq[None, :]  # [S, 64]
    cs = np.cos(ang).astype(np.float32).T          # [64, S]
    sn = np.sin(ang).astype(np.float32).T
    cs = np.ascontiguousarray(np.concatenate([cs, cs], axis=0))  # permuted layout
    sn = np.ascontiguousarray(np.concatenate([sn, sn], axis=0))

    masks = np.zeros((P, 2 * 512), np.float32)
    kk = np.arange(P)[:, None]
    qq = np.arange(512)[None, :]
    for jj in range(2):
        masks[:, 512 * jj:512 * (jj + 1)] = (qq >= kk + P * jj).astype(np.float32)

    ones = np.ones((P, 1), np.float32)
    ident = np.eye(P, dtype=np.float32)

    xT = np.ascontiguousarray(x.reshape(T, DM).T)
    woT = np.ascontiguousarray(wo.T)
    if WO_BF16:
        woT = woT.astype(ml_dtypes.bfloat16)

    shared = {"xT": xT, "woT": woT, "cs": cs, "sn": sn,
              "masks": masks, "ones": ones, "ident": ident}

    in_maps = []
    for c in range(N_CORES):
        wq_c = wq[512 * c:512 * (c + 1)].reshape(HQ, HD, DM)[:, _PERM, :].reshape(512, DM)
        wk_c = wk[HD * c:HD * (c + 1)][_PERM]
        wv_c = wv[HD * c:HD * (c + 1)]
        Wc = np.concatenate([wq_c, wk_c, wv_c], axis=0)        # [768, DM]
        wT = np.ascontiguousarray(Wc.T).reshape(KO, P, MW)
        in_maps.append({**shared, "wT": wT})
    return in_maps


def kernel(x, wq, wk, wv, wo):
    x = np.asarray(x, np.float32)
    wq = np.asarray(wq, np.float32)
    wk = np.asarray(wk, np.float32)
    wv = np.asarray(wv, np.float32)
    wo = np.asarray(wo, np.float32)

    if "nc" not in _CACHE:
        _CACHE["nc"] = _build_nc()
    nc = _CACHE["nc"]

    in_maps = _host_prep(x, wq, wk, wv, wo)
    res = run_bass_kernel_spmd(nc, in_maps, list(range(N_CORES))).results

    out = np.empty((B, S, DM), np.float32)
    new_k = np.empty((B, NKV, S, HD), np.float32)
    new_v = np.empty((B, NKV, S, HD), np.float32)
    for c in range(N_CORES):
        oc = res[c]["out_c"]                       # [2*TB, DM]
        for b in range(B):
            out[b, TB * c:TB * (c + 1), :] = oc[TB * b:TB * (b + 1)]
        nk = res[c]["newkT"]                       # [B, 128, S] permuted rows
        nv = res[c]["newvT"]
        for b in range(B):
            new_k[b, c][:, _PERM] = nk[b].T
            new_v[b, c] = nv[b].T
    return out, new_k, new_v


# revision 12
# speedup vs baseline: 1.2256x; 1.2256x over previous
"""Trainium2 Bass kernel for GQA attention block (nn_Attention_52115133170131).

Strategy (8 NeuronCores, tensor-parallel over heads):
  - Core c owns q-heads 4c..4c+3 and kv-head c (wq/wk/wv column-sharded,
    head-dim pairs permuted evens-first so RoPE works on contiguous
    partition halves).
  - Per core: fused QKV projection (K=4096 float32r matmuls), RoPE on q/k,
    causal attention in transposed (feature-major) space: exp on ScalarE,
    softmax denominator via a ones-column matmul accumulated in PSUM,
    PV accumulated in PSUM, normalization via reciprocal + partition
    broadcast.
  - Per-batch AllToAll redistributes attention output from head-sharded to
    token-sharded (bf16 payload); each core then runs the full wo matmul
    for its 256 tokens of that batch (full contraction locally, no partial
    sums).  Batch-0's AllToAll and wo sweep overlap batch-1's attention.
  - Host gathers: concat token slices (out), stack kv-head shards
    (new_k/new_v; device writes them feature-major, host transposes).

QKV/attention matmuls run in float32r (full PE rate at N>=256, ~1.5e-4
rel err).  The wo projection runs in bf16 (WO_BF16) which only affects
`out` (~1e-3 rel err) and halves the collective payload.
"""
from contextlib import ExitStack

import numpy as np

import concourse.bacc as bacc
import concourse.mybir as mybir
import concourse.tile as tile
from concourse.bass_utils import run_bass_kernel_spmd

# problem constants
N_CORES = 8
B = 2
S = 2048
DM = 4096            # model dim
NH = 32              # query heads
NKV = 8              # kv heads
HD = 128             # head dim
HQ = NH // N_CORES   # q heads per core = 4
T = B * S            # 4096 tokens
TB = S // N_CORES    # tokens per core per batch = 256
ROPE_THETA = 10000.0
SCALE = float(1.0 / np.sqrt(np.float32(HD)))

P = 128
KO = DM // P         # 32 contraction k-tiles
NTC = T // 512       # 8 projection t-chunks
MW = HQ * HD + 2 * HD  # 768 fused qkv out dim per core

F32R = mybir.dt.float32r
F32 = mybir.dt.float32
BF16 = mybir.dt.bfloat16

WO_BF16 = True       # wo matmul + AllToAll payload in bf16
SPLIT = True         # per-batch A2A overlapped with attention/wo
XT_BUFS = 12
QQ_BUFS = 3
EE_BUFS = 5
WT_BUFS = 6

_PERM = np.concatenate([np.arange(0, HD, 2), np.arange(1, HD, 2)])  # evens-first

_CACHE = {}


def _build_nc(reps=1):
    a2a_dt = BF16 if WO_BF16 else F32R

    nc = bacc.Bacc("TRN2", target_bir_lowering=False)

    xT_d = nc.dram_tensor("xT", [DM, T], F32R, kind="ExternalInput")
    wT_d = nc.dram_tensor("wT", [KO, P, MW], F32R, kind="ExternalInput")
    woT_d = nc.dram_tensor("woT", [DM, DM], a2a_dt, kind="ExternalInput")
    cs_d = nc.dram_tensor("cs", [P, S], F32, kind="ExternalInput")
    sn_d = nc.dram_tensor("sn", [P, S], F32, kind="ExternalInput")
    masks_d = nc.dram_tensor("masks", [P, 2 * 512], F32R, kind="ExternalInput")
    ones_d = nc.dram_tensor("ones", [P, 1], F32R, kind="ExternalInput")
    ident_d = nc.dram_tensor("ident", [P, P], F32R, kind="ExternalInput")

    out_d = nc.dram_tensor("out_c", [2 * TB, DM], F32, kind="ExternalOutput")
    newk_d = nc.dram_tensor("newkT", [B, P, S], F32, kind="ExternalOutput")
    newv_d = nc.dram_tensor("newvT", [B, P, S], F32, kind="ExternalOutput")

    qsp_d = nc.dram_tensor("qsp", [B, HQ, P, S], F32R)
    a2a_in_d = [nc.dram_tensor(f"a2a_in{b}", [N_CORES, 512, TB], a2a_dt)
                for b in range(B)]
    a2a_out_d = [nc.dram_tensor(f"a2a_out{b}", [N_CORES, 512, TB], a2a_dt)
                 for b in range(B)]

    with tile.TileContext(nc) as tc:
        with ExitStack() as consts:
            cpool = consts.enter_context(tc.tile_pool(name="consts", bufs=1))
            cs = cpool.tile([P, S], F32)
            sn = cpool.tile([P, S], F32)
            masks = cpool.tile([P, 2 * 512], F32R)
            ones = cpool.tile([P, 1], F32R)
            ident = cpool.tile([P, P], F32R)
            for t, d in [(cs, cs_d), (sn, sn_d), (masks, masks_d),
                         (ones, ones_d), (ident, ident_d)]:
                nc.sync.dma_start(out=t[:], in_=d[:])

            # resident attention operands
            kvpool = consts.enter_context(tc.tile_pool(name="kv", bufs=1))
            kT_rot = [kvpool.tile([P, S], F32R, name=f"kTrot{b}") for b in range(B)]
            v_tok = [[kvpool.tile([P, P], F32R, name=f"vtok{b}_{j}")
                      for j in range(S // P)] for b in range(B)]

            for _rep in range(reps):
                _emit_body(nc, tc, cs, sn, masks, ones, ident, kT_rot, v_tok,
                           xT_d, wT_d, woT_d, out_d, newk_d, newv_d,
                           qsp_d, a2a_in_d, a2a_out_d, a2a_dt)
    nc.compile()
    return nc


def _emit_body(nc, tc, cs, sn, masks, ones, ident, kT_rot, v_tok,
               xT_d, wT_d, woT_d, out_d, newk_d, newv_d,
               qsp_d, a2a_in_d, a2a_out_d, a2a_dt):
    # ---------------- phase B: fused qkv projection + rope ------------------
    with ExitStack() as ph:
        wpool = ph.enter_context(tc.tile_pool(name="wq", bufs=1))
        W = []
        for ko in range(KO):
            w = wpool.tile([P, MW], F32R, name=f"w{ko}")
            nc.sync.dma_start(out=w[:], in_=wT_d[ko])
            W.append(w)

        xpool = ph.enter_context(tc.tile_pool(name="xt", bufs=XT_BUFS))
        pps = ph.enter_context(tc.tile_pool(name="projps", bufs=1, space="PSUM"))
        tpps = ph.enter_context(tc.tile_pool(name="tpps", bufs=2, space="PSUM"))
        ev = ph.enter_context(tc.tile_pool(name="evict", bufs=2))

        def rope(dst, src, col0, n):
            # dst[0:64]   = src[0:64]*cs - src[64:128]*sn   (on dst lanes)
            # dst[64:128] = src[0:64]*sn + src[64:128]*cs
            A = ev.tile([P, n], F32, name="ropeA")
            Bt = ev.tile([P, n], F32, name="ropeB")
            Bs = ev.tile([P, n], F32, name="ropeBs")
            nc.vector.tensor_mul(A[:], src[:], cs[:, col0:col0 + n])
            nc.vector.tensor_mul(Bt[:], src[:], sn[:, col0:col0 + n])
            nc.sync.dma_start(out=Bs[0:64, :], in_=Bt[64:128, :])
            nc.sync.dma_start(out=Bs[64:128, :], in_=Bt[0:64, :])
            nc.vector.tensor_sub(dst[0:64, :], A[0:64, :], Bs[0:64, :])
            nc.vector.tensor_add(dst[64:128, :], Bs[64:128, :], A[64:128, :])

        for tcn in range(NTC):
            b, tci = tcn // 4, tcn % 4
            col0 = 512 * tci
            ps = [pps.tile([P, 512], F32, name=f"pp{m}") for m in range(6)]
            for ko in range(KO):
                xt = xpool.tile([P, 512], F32R, name="xt")
                nc.sync.dma_start(
                    out=xt[:], in_=xT_d[P * ko:P * (ko + 1),
                                        512 * tcn:512 * (tcn + 1)])
                for m in range(6):
                    nc.tensor.matmul(
                        ps[m][:], W[ko][:, P * m:P * (m + 1)], xt[:],
                        start=(ko == 0), stop=(ko == KO - 1))
            # q heads: rope -> spill
            for m in range(HQ):
                qrt = ev.tile([P, 512], F32R, name="qrt")
                rope(qrt, ps[m], col0, 512)
                nc.sync.dma_start(out=qsp_d[b, m, :, col0:col0 + 512], in_=qrt[:])
            # k: new_k (pre-rope) + rope into resident kT_rot
            nkst = ev.tile([P, 512], F32, name="nkst")
            nc.vector.tensor_copy(nkst[:], ps[4][:])
            nc.sync.dma_start(out=newk_d[b][:, col0:col0 + 512], in_=nkst[:])
            rope(kT_rot[b][:, col0:col0 + 512], ps[4], col0, 512)
            # v: new_v + transpose to token-major tiles
            vst = ev.tile([P, 512], F32R, name="vst")
            nc.vector.tensor_copy(vst[:], ps[5][:])
            nvst = ev.tile([P, 512], F32, name="nvst")
            nc.vector.tensor_copy(nvst[:], ps[5][:])
            nc.sync.dma_start(out=newv_d[b][:, col0:col0 + 512], in_=nvst[:])
            for j in range(4):
                tp = tpps.tile([P, P], F32R, name="tp")
                nc.tensor.transpose(tp[:], vst[:, P * j:P * (j + 1)], ident[:])
                nc.vector.tensor_copy(v_tok[b][4 * tci + j][:], tp[:])

    # ---------------- phase C/D: attention, per-batch A2A, wo ---------------
    def attn_batch(ph, b):
        # software-pipelined inner loop: PV/den for tile j are emitted SKEW
        # scores-tiles later, so the exp (ACT) + mask (DVE) latency of et(j)
        # is covered by PE work instead of stalling the PE queue.
        SKEW = 2
        qpool, spool, opool, dpool, epool, npool = ph
        for h in range(HQ):
            for qc in range(4):
                q0 = 512 * qc
                ktmax = 4 * (qc + 1)
                qt = qpool.tile([P, 512], F32R, name="qt")
                nc.sync.dma_start(out=qt[:], in_=qsp_d[b, h, :, q0:q0 + 512])
                otps = opool.tile([P, 512], F32, name="otps")
                dnps = dpool.tile([1, 512], F32, name="dnps")

                def tile_window(j):
                    # causal: tile j only needs q columns >= 128*jj; clamp
                    # the window to >=256 cols (f32r full-rate floor)
                    jj = j - 4 * qc
                    if jj >= 1:
                        npr = max(512 - 128 * jj, 256)
                        n0 = 512 - npr
                    else:
                        n0, npr = 0, 512
                    return jj, n0, npr

                ets = {}

                def emit_scores(j):
                    jj, n0, npr = tile_window(j)
                    sps = spool.tile([P, 512], F32, name="sps")
                    nc.tensor.matmul(sps[:, n0:n0 + npr],
                                     kT_rot[b][:, P * j:P * (j + 1)],
                                     qt[:, n0:n0 + npr], start=True, stop=True)
                    et = epool.tile([P, 512], F32R, name="et")
                    nc.scalar.activation(et[:, n0:n0 + npr], sps[:, n0:n0 + npr],
                                         mybir.ActivationFunctionType.Exp,
                                         scale=SCALE)
                    if jj >= 0:
                        o = jj - n0 // P
                        nc.vector.tensor_mul(
                            et[:, n0:n0 + npr], et[:, n0:n0 + npr],
                            masks[:, 512 * o:512 * o + npr])
                    ets[j] = (et, n0, npr)

                def emit_pv_den(j):
                    et, n0, npr = ets.pop(j)
                    nc.tensor.matmul(otps[:, n0:n0 + npr], v_tok[b][j][:],
                                     et[:, n0:n0 + npr],
                                     start=(j == 0), stop=(j == ktmax - 1))
                    nc.tensor.matmul(dnps[:, n0:n0 + npr], ones[:],
                                     et[:, n0:n0 + npr],
                                     start=(j == 0), stop=(j == ktmax - 1))

                for j in range(ktmax):
                    emit_scores(j)
                    if j >= SKEW:
                        emit_pv_den(j - SKEW)
                for j in range(max(0, ktmax - SKEW), ktmax):
                    emit_pv_den(j)

                rc = npool.tile([1, 512], F32, name="rc")
                nc.vector.reciprocal(rc[:], dnps[:])
                rb = npool.tile([P, 512], F32, name="rb")
                nc.gpsimd.partition_broadcast(rb[:], rc[:])
                at = npool.tile([P, 512], a2a_dt, name="at")
                nc.vector.tensor_mul(at[:], otps[:], rb[:])
                nc.sync.dma_start(
                    out=a2a_in_d[b][2 * qc, HD * h:HD * (h + 1), :],
                    in_=at[:, 0:TB])
                nc.sync.dma_start(
                    out=a2a_in_d[b][2 * qc + 1, HD * h:HD * (h + 1), :],
                    in_=at[:, TB:512])

    def wo_batch(ph, batches):
        # one woT sweep shared by all listed batches: each weight tile feeds
        # 4 matmuls per batch, halving woT DMA when both batches are ready
        lpool, wopool, wps, oev = ph
        lt = {}
        for b in batches:
            for f in range(KO):
                t_ = lpool.tile([P, TB], a2a_dt, name=f"lt{b}_{f}")
                nc.sync.dma_start(
                    out=t_[:],
                    in_=a2a_out_d[b][f // 4, P * (f % 4):P * (f % 4 + 1), :])
                lt[b, f] = t_
        for n in range(DM // 512):
            # wt tiles for this n-block stay resident so every batch reuses
            # them (PSUM: only 2 accumulator banks, batches run back-to-back)
            wts = [None] * KO
            for bi, b in enumerate(batches):
                pso = [wps.tile([P, 512], F32, name=f"wps{t_}")
                       for t_ in range(TB // P)]
                for f in range(KO):
                    if bi == 0:
                        wt = wopool.tile([P, 512], a2a_dt, name="wt")
                        nc.sync.dma_start(
                            out=wt[:], in_=woT_d[P * f:P * (f + 1),
                                                 512 * n:512 * (n + 1)])
                        wts[f] = wt
                    for t_ in range(TB // P):
                        nc.tensor.matmul(
                            pso[t_][:], lt[b, f][:, P * t_:P * (t_ + 1)],
                            wts[f][:], start=(f == 0), stop=(f == KO - 1))
                for t_ in range(TB // P):
                    ot = oev.tile([P, 512], F32, name="ot")
                    nc.vector.tensor_copy(ot[:], pso[t_][:])
                    r0 = TB * b + P * t_
                    nc.sync.dma_start(
                        out=out_d[r0:r0 + P, 512 * n:512 * (n + 1)], in_=ot[:])

    rg = [list(range(N_CORES))]

    def a2a(b):
        nc.gpsimd.collective_compute(
            "AllToAll", mybir.AluOpType.bypass,
            ins=[a2a_in_d[b][:]], outs=[a2a_out_d[b][:]], replica_groups=rg)

    if SPLIT:
        with ExitStack() as ph:
            attn_pools = (
                ph.enter_context(tc.tile_pool(name="qq", bufs=QQ_BUFS)),
                ph.enter_context(tc.tile_pool(name="sps", bufs=3, space="PSUM")),
                ph.enter_context(tc.tile_pool(name="ops", bufs=2, space="PSUM")),
                ph.enter_context(tc.tile_pool(name="dps", bufs=1, space="PSUM")),
                ph.enter_context(tc.tile_pool(name="ee", bufs=EE_BUFS)),
                ph.enter_context(tc.tile_pool(name="nrm", bufs=2)),
            )
            wo_pools = (
                ph.enter_context(tc.tile_pool(name="lt", bufs=1)),
                ph.enter_context(tc.tile_pool(name="wo", bufs=WT_BUFS)),
                ph.enter_context(tc.tile_pool(name="wops", bufs=1, space="PSUM")),
                ph.enter_context(tc.tile_pool(name="oev", bufs=3)),
            )
            attn_batch(attn_pools, 0)
            a2a(0)
            attn_batch(attn_pools, 1)
            wo_batch(wo_pools, [0])
            a2a(1)
            wo_batch(wo_pools, [1])
    else:
        # unified wo: both batches share one woT sweep (half the wo DMA);
        # attention pools close before the wo pools open so wo gets 4 PSUM
        # banks without fighting the attention pools
        with ExitStack() as ph:
            attn_pools = (
                ph.enter_context(tc.tile_pool(name="qq", bufs=QQ_BUFS)),
                ph.enter_context(tc.tile_pool(name="sps", bufs=3, space="PSUM")),
                ph.enter_context(tc.tile_pool(name="ops", bufs=2, space="PSUM")),
                ph.enter_context(tc.tile_pool(name="dps", bufs=1, space="PSUM")),
                ph.enter_context(tc.tile_pool(name="ee", bufs=EE_BUFS)),
                ph.enter_context(tc.tile_pool(name="nrm", bufs=2)),
            )
            attn_batch(attn_pools, 0)
            a2a(0)
            attn_batch(attn_pools, 1)
            a2a(1)
        with ExitStack() as ph:
            wo_pools = (
                ph.enter_context(tc.tile_pool(name="lt", bufs=1)),
                ph.enter_context(tc.tile_pool(name="wo", bufs=KO + 4)),
                ph.enter_context(tc.tile_pool(name="wops", bufs=1, space="PSUM")),
                ph.enter_context(tc.tile_pool(name="oev", bufs=3)),
            )
            wo_batch(wo_pools, [0, 1])


def _host_prep(x, wq, wk, wv, wo):
    import ml_dtypes

    # rope tables exactly as the reference computes them (fp32 ops)
    inv_freq = (1.0 / (ROPE_THETA ** (np.arange(0, HD, 2, dtype=np.float32) / HD))
                ).astype(np.float32)
    ang = np.arange(S, dtype=np.float32)[:, None] * inv_freq[None, :]  # [S, 64]
    cs = np.cos(ang).astype(np.float32).T          # [64, S]
    sn = np.sin(ang).astype(np.float32).T
    cs = np.ascontiguousarray(np.concatenate([cs, cs], axis=0))  # permuted layout
    sn = np.ascontiguousarray(np.concatenate([sn, sn], axis=0))

    masks = np.zeros((P, 2 * 512), np.float32)
    kk = np.arange(P)[:, None]
    qq = np.arange(512)[None, :]
    for jj in range(2):
        masks[:, 512 * jj:512 * (jj + 1)] = (qq >= kk + P * jj).astype(np.float32)

    ones = np.ones((P, 1), np.float32)
    ident = np.eye(P, dtype=np.float32)

    xT = np.ascontiguousarray(x.reshape(T, DM).T)
    woT = np.ascontiguousarray(wo.T)
    if WO_BF16:
        woT = woT.astype(ml_dtypes.bfloat16)

    shared = {"xT": xT, "woT": woT, "cs": cs, "sn": sn,
              "masks": masks, "ones": ones, "ident": ident}

    in_maps = []
    for c in range(N_CORES):
        wq_c = wq[512 * c:512 * (c + 1)].reshape(HQ, HD, DM)[:, _PERM, :].reshape(512, DM)
        wk_c = wk[HD * c:HD * (c + 1)][_PERM]
        wv_c = wv[HD * c:HD * (c + 1)]
        Wc = np.concatenate([wq_c, wk_c, wv_c], axis=0)        # [768, DM]
        wT = np.ascontiguousarray(Wc.T).reshape(KO, P, MW)
        in_maps.append({**shared, "wT": wT})
    return in_maps


def kernel(x, wq, wk, wv, wo):
    x = np.asarray(x, np.float32)
    wq = np.asarray(wq, np.float32)
    wk = np.asarray(wk, np.float32)
    wv = np.asarray(wv, np.float32)
    wo = np.asarray(wo, np.float32)

    if "nc" not in _CACHE:
        _CACHE["nc"] = _build_nc()
    nc = _CACHE["nc"]

    in_maps = _host_prep(x, wq, wk, wv, wo)
    res = run_bass_kernel_spmd(nc, in_maps, list(range(N_CORES))).results

    out = np.empty((B, S, DM), np.float32)
    new_k = np.empty((B, NKV, S, HD), np.float32)
    new_v = np.empty((B, NKV, S, HD), np.float32)
    for c in range(N_CORES):
        oc = res[c]["out_c"]                       # [2*TB, DM]
        for b in range(B):
            out[b, TB * c:TB * (c + 1), :] = oc[TB * b:TB * (b + 1)]
        nk = res[c]["newkT"]                       # [B, 128, S] permuted rows
        nv = res[c]["newvT"]
        for b in range(B):
            new_k[b, c][:, _PERM] = nk[b].T
            new_v[b, c] = nv[b].T
    return out, new_k, new_v
